# revision 17
# baseline (speedup 1.0000x reference)
"""Causal full attention with learnable (L,L) additive bias, on 8 trn2 cores.

Problem: B=4, L=2048, H=8, E=64.
  scores = einsum("blhe,bshe->bhls", q, k) + causal_mask[None,None]
  scores = where(attn_mask, -inf, scores)
  A = softmax(scale * scores, -1);  out = einsum("bhls,bshd->blhd", A, v)

Sharding: core c gets head c (all 4 batches) -> 4 independent (b,h)
attention problems per core, no cross-core communication.

Device algorithm (per core, per pair p=(b, h=core)):
  S^T[k,q] = (K^T)^T_block @ (scale*Q)^T   (k on partitions; no transpose needed
                                            later because A^T is exactly the
                                            matmul layout the AV product wants)
  E = exp(S^T)                 fp16        (ScalarE, PSUM->SBUF)
  R = E * expb^T[k,q]          fp16        (DVE; expb = exp(scale*bias), 0 where
                                            masked -> folds bias AND mask)
  acc[e,q] += V_aug[kblk]^T @ R            (V_aug has a ones column -> row 64 of
                                            acc accumulates the softmax denom)
Host epilogue: out = (acc[:64]/acc[64]).T per pair.

Block skip-list is derived from the actual attn_mask/causal_mask content, so
the kernel is correct for any mask; for the causal mask it skips ~half the
blocks.
"""

import math

import numpy as np

import concourse.bass as bass
import concourse.mybir as mybir
import concourse.tile as tile
from concourse import bacc
from concourse.bass_utils import run_bass_kernel_spmd

B, L, H, E = 4, 2048, 8, 64
NCORES = 8
PAIRS = B  # (b, h=core) pairs per core
SCALE = 1.0 / math.sqrt(E)

KB = 128          # k-block (PSUM partition dim of S^T)
NKB = L // KB     # 16
QC = 512          # q-chunk (matmul free dim / PSUM bank width)
NQC = L // QC     # 4
JG = 2            # k-blocks fused per exp/mul instruction (FD = JG*QC)
NG = NKB // JG    # 8 groups
VW = 66           # padded V_aug block width (64 + ones + pad for 4B alignment)

DT = mybir.dt.float16
NPDT = np.float16

# test harness hooks
TRACE = False
LAST = {}
REPS = 1  # bench-only: repeat the whole compute inside the program (For_i)
VARIANT = "base"  # bench-only engine-scaling probes: base|act2x|dve2x|pe2x

_cache = {}


def _plan(expbT):
    """Per group g=(2g, 2g+1): inclusive qc range that contains any nonzero
    expb column, or None if the whole group is masked out."""
    nz = np.zeros((NKB, NQC), dtype=bool)
    for j in range(NKB):
        blk = expbT[j * KB : (j + 1) * KB]
        for qc in range(NQC):
            nz[j, qc] = np.any(blk[:, qc * QC : (qc + 1) * QC])
    ranges = []
    for g in range(NG):
        v = nz[2 * g] | nz[2 * g + 1]
        idx = np.flatnonzero(v)
        ranges.append((int(idx[0]), int(idx[-1])) if len(idx) else None)
    return tuple(ranges)


def _expb_cols(ranges):
    return sum((qce - qcs + 1) * JG * QC for r in ranges if r is not None
               for qcs, qce in [r])


def _build(ranges):
    """Build the SPMD Bass program for one core (identical on all cores)."""
    expb_cols = _expb_cols(ranges)
    nc = bacc.Bacc("TRN2", target_bir_lowering=False, debug=False)

    qt = [nc.dram_tensor(f"qt{p}", [64, L], DT, kind="ExternalInput").ap()
          for p in range(PAIRS)]
    kt = [nc.dram_tensor(f"kt{p}", [64, L], DT, kind="ExternalInput").ap()
          for p in range(PAIRS)]
    vaug = nc.dram_tensor("vaug", [128, PAIRS * NKB * VW], DT,
                          kind="ExternalInput").ap()
    expb = nc.dram_tensor("expb", [128, expb_cols], DT,
                          kind="ExternalInput").ap()
    ot = nc.dram_tensor("ot", [PAIRS, E + 1, L], mybir.dt.float32,
                        kind="ExternalOutput").ap()

    # group g's packed-expb column offset
    goff = {}
    off = 0
    for g, r in enumerate(ranges):
        if r is None:
            continue
        goff[g] = off
        off += (r[1] - r[0] + 1) * JG * QC

    f32 = mybir.dt.float32
    with tile.TileContext(nc) as tc:
        with (
            tc.tile_pool(name="const", bufs=1) as const_pool,
            tc.tile_pool(name="work", bufs=3) as work_pool,
            tc.tile_pool(name="ostage", bufs=2) as out_pool,
            tc.tile_pool(name="spsum", bufs=3, space="PSUM") as s_pool,
            tc.tile_pool(name="apsum", bufs=2, space="PSUM") as acc_pool,
        ):
            qt_sb = [const_pool.tile([64, L], DT, tag=f"qt{p}", name=f"qt{p}_sb")
                     for p in range(PAIRS)]
            kt_sb = [const_pool.tile([64, L], DT, tag=f"kt{p}", name=f"kt{p}_sb")
                     for p in range(PAIRS)]
            vaug_sb = const_pool.tile([128, PAIRS * NKB * VW], DT, tag="vaug")
            expb_sb = const_pool.tile([128, expb_cols], DT, tag="expb")

            nc.sync.dma_start(qt_sb[0][:], qt[0])
            nc.sync.dma_start(kt_sb[0][:], kt[0])
            nc.sync.dma_start(vaug_sb[:], vaug)
            # split the big bias load so compute can start on early chunks
            nsplit = 8
            step = -(-expb_cols // nsplit)
            step += step % 2  # keep 4B alignment
            for a in range(0, expb_cols, step):
                b = min(a + step, expb_cols)
                nc.sync.dma_start(expb_sb[:, a:b], expb[:, a:b])
            for p in range(1, PAIRS):
                nc.sync.dma_start(qt_sb[p][:], qt[p])
                nc.sync.dma_start(kt_sb[p][:], kt[p])

            def compute_ilv():
                # qc outer, group middle, pair inner: 4 independent
                # per-pair dependency chains in flight at all times.
                for qc in range(NQC):
                    gs = [g for g in range(NG)
                          if ranges[g] is not None
                          and ranges[g][0] <= qc <= ranges[g][1]]
                    if not gs:
                        continue
                    accs = [acc_pool.tile([E + 1, QC], f32, tag=f"acc{p}",
                                          bufs=1, name=f"acc{p}")
                            for p in range(PAIRS)]
                    nmm = JG * len(gs)
                    mms = [0] * PAIRS
                    for g in gs:
                        boff = goff[g] + (qc - ranges[g][0]) * JG * QC
                        for p in range(PAIRS):
                            s_t = s_pool.tile([128, JG * QC], f32, tag="s",
                                              bufs=2, name="s_t")
                            for t in range(JG):
                                j = JG * g + t
                                nc.tensor.matmul(
                                    s_t[:, t * QC : (t + 1) * QC],
                                    kt_sb[p][:, j * KB : (j + 1) * KB],
                                    qt_sb[p][:, qc * QC : (qc + 1) * QC],
                                    start=True, stop=True,
                                )
                            e_t = work_pool.tile([128, JG * QC], DT, tag="exp",
                                                 bufs=6, name="e_t")
                            nc.scalar.activation(
                                e_t[:], s_t[:], mybir.ActivationFunctionType.Exp
                            )
                            r_t = work_pool.tile([128, JG * QC], DT, tag="rhs",
                                                 bufs=6, name="r_t")
                            nc.vector.tensor_mul(
                                r_t[:], e_t[:], expb_sb[:, boff : boff + JG * QC]
                            )
                            for t in range(JG):
                                j = JG * g + t
                                voff = (p * NKB + j) * VW
                                nc.tensor.matmul(
                                    accs[p][:],
                                    vaug_sb[:, voff : voff + E + 1],
                                    r_t[:, t * QC : (t + 1) * QC],
                                    start=(mms[p] == 0),
                                    stop=(mms[p] == nmm - 1),
                                )
                                mms[p] += 1
                    for p in range(PAIRS):
                        st = out_pool.tile([E + 1, QC], f32, tag="st",
                                           name="st")
                        nc.vector.tensor_copy(st[:], accs[p][:])
                        nc.sync.dma_start(ot[p][:, qc * QC : (qc + 1) * QC],
                                          st[:])

            def compute_duo():
                # two independent pair-chains (A: pairs 0/1, B: pairs 2/3)
                # interleaved; keeps s_pool triple-buffered (2+3*2=8 banks).
                wb = 8 if VARIANT == "duo8" else 3
                for half in range(2):
                    pA, pB = 2 * half, 2 * half + 1
                    for qc in range(NQC):
                        gs = [g for g in range(NG)
                              if ranges[g] is not None
                              and ranges[g][0] <= qc <= ranges[g][1]]
                        if not gs:
                            continue
                        accs = {p: acc_pool.tile([E + 1, QC], f32,
                                                 tag=f"acc{p % 2}", bufs=1,
                                                 name=f"acc{p}")
                                for p in (pA, pB)}
                        nmm = JG * len(gs)
                        mms = {pA: 0, pB: 0}
                        for g in gs:
                            boff = goff[g] + (qc - ranges[g][0]) * JG * QC
                            for p in (pA, pB):
                                s_t = s_pool.tile([128, JG * QC], f32,
                                                  tag="s", bufs=3, name="s_t")
                                for t in range(JG):
                                    j = JG * g + t
                                    nc.tensor.matmul(
                                        s_t[:, t * QC : (t + 1) * QC],
                                        kt_sb[p][:, j * KB : (j + 1) * KB],
                                        qt_sb[p][:, qc * QC : (qc + 1) * QC],
                                        start=True, stop=True,
                                    )
                                e_t = work_pool.tile([128, JG * QC], DT,
                                                     tag="exp", bufs=wb,
                                                     name="e_t")
                                nc.scalar.activation(
                                    e_t[:], s_t[:],
                                    mybir.ActivationFunctionType.Exp)
                                r_t = work_pool.tile([128, JG * QC], DT,
                                                     tag="rhs", bufs=wb,
                                                     name="r_t")
                                nc.vector.tensor_mul(
                                    r_t[:], e_t[:],
                                    expb_sb[:, boff : boff + JG * QC])
                                for t in range(JG):
                                    j = JG * g + t
                                    voff = (p * NKB + j) * VW
                                    nc.tensor.matmul(
                                        accs[p][:],
                                        vaug_sb[:, voff : voff + E + 1],
                                        r_t[:, t * QC : (t + 1) * QC],
                                        start=(mms[p] == 0),
                                        stop=(mms[p] == nmm - 1),
                                    )
                                    mms[p] += 1
                        for p in (pA, pB):
                            st = out_pool.tile([E + 1, QC], f32, tag="st",
                                               bufs=4, name="st")
                            nc.vector.tensor_copy(st[:], accs[p][:])
                            nc.sync.dma_start(
                                ot[p][:, qc * QC : (qc + 1) * QC], st[:])

            def compute():
              if VARIANT in ("duo", "duo8"):
                  compute_duo()
                  return
              if VARIANT == "empty":
                  st = out_pool.tile([E + 1, QC], f32, tag="st", name="st0")
                  nc.vector.tensor_copy(st[:], expb_sb[: E + 1, :QC])
                  return
              if VARIANT == "ilv":
                  compute_ilv()
                  return
              npairs = 2 if VARIANT == "half" else PAIRS
              for p in range(npairs):
                qts = qt_sb[p]
                kts = kt_sb[p]
                lo, hi = 0, 64
                for qc in range(NQC):
                    gs = [g for g in range(NG)
                          if ranges[g] is not None
                          and ranges[g][0] <= qc <= ranges[g][1]]
                    if not gs:
                        continue
                    acc = acc_pool.tile([E + 1, QC], f32, tag="acc")
                    nmm = JG * len(gs) * (2 if VARIANT == "pe2x" else 1)
                    mm = 0
                    for g in gs:
                        s_t = s_pool.tile([128, JG * QC], f32, tag="s")
                        for t in range(JG):
                            j = JG * g + t
                            for _dup in range(2 if VARIANT == "pe2x" else 1):
                                nc.tensor.matmul(
                                    s_t[:, t * QC : (t + 1) * QC],
                                    kts[lo:hi, j * KB : (j + 1) * KB],
                                    qts[lo:hi, qc * QC : (qc + 1) * QC],
                                    start=True, stop=True,
                                )
                        e_t = work_pool.tile([128, JG * QC], DT, tag="exp")
                        nc.scalar.activation(
                            e_t[:], s_t[:], mybir.ActivationFunctionType.Exp
                        )
                        if VARIANT == "act2x":
                            e_t2 = work_pool.tile([128, JG * QC], DT, tag="exp2")
                            nc.scalar.copy(e_t2[:], e_t[:])
                            e_t = e_t2
                        r_t = work_pool.tile([128, JG * QC], DT, tag="rhs")
                        boff = goff[g] + (qc - ranges[g][0]) * JG * QC
                        nc.vector.tensor_mul(
                            r_t[:], e_t[:], expb_sb[:, boff : boff + JG * QC]
                        )
                        if VARIANT == "dve2x":
                            r_t2 = work_pool.tile([128, JG * QC], DT, tag="rhs2")
                            nc.vector.tensor_mul(
                                r_t2[:], r_t[:], expb_sb[:, boff : boff + JG * QC]
                            )
                            r_t = r_t2
                        for t in range(JG):
                            j = JG * g + t
                            voff = (p * NKB + j) * VW
                            for dup in range(2 if VARIANT == "pe2x" else 1):
                                nc.tensor.matmul(
                                    acc[:],
                                    vaug_sb[:, voff : voff + E + 1],
                                    r_t[:, t * QC : (t + 1) * QC],
                                    start=(mm == 0), stop=(mm == nmm - 1),
                                )
                                mm += 1
                    st = out_pool.tile([E + 1, QC], f32, tag="st")
                    nc.vector.tensor_copy(st[:], acc[:])
                    nc.sync.dma_start(ot[p][:, qc * QC : (qc + 1) * QC], st[:])

            def compute_n():
                compute()
                if VARIANT == "dbl":
                    compute()

            if REPS > 1:
                hints = (mybir.EngineType.PE, mybir.EngineType.Activation,
                         mybir.EngineType.DVE, mybir.EngineType.SP)
                with tc.For_i(0, REPS, 1, hint_engines=hints):
                    compute_n()
            else:
                compute_n()
    nc.compile()
    return nc


def kernel(queries, keys, values, attn_mask, causal_mask):
    queries = np.asarray(queries, dtype=np.float32)
    keys = np.asarray(keys, dtype=np.float32)
    values = np.asarray(values, dtype=np.float32)
    attn_mask = np.asarray(attn_mask).astype(bool).reshape(L, L)
    causal_mask = np.asarray(causal_mask, dtype=np.float32)
    assert queries.shape == (B, L, H, E)

    # exp of the scaled additive bias, 0 where masked; [k, q] orientation
    expbT = np.where(attn_mask, 0.0, np.exp(SCALE * causal_mask)).T
    expbT = np.ascontiguousarray(expbT, dtype=NPDT)

    ranges = _plan(expbT != 0)
    if ranges not in _cache:
        _cache[ranges] = _build(ranges)
    nc = _cache[ranges]

    # pack expb: group-major, then qc, then [j0-chunk | j1-chunk]
    chunks = []
    for g, r in enumerate(ranges):
        if r is None:
            continue
        for qc in range(r[0], r[1] + 1):
            for t in range(JG):
                j = JG * g + t
                chunks.append(
                    expbT[j * KB : (j + 1) * KB, qc * QC : (qc + 1) * QC])
    expb_packed = np.ascontiguousarray(np.concatenate(chunks, axis=1))

    in_maps = []
    for c in range(NCORES):
        qts, kts = [], []
        va = np.zeros((128, PAIRS * NKB * VW), dtype=NPDT)
        for p in range(PAIRS):
            qts.append((queries[p, :, c, :].T * SCALE).astype(NPDT))
            kts.append(keys[p, :, c, :].T.astype(NPDT))
            vp = values[p, :, c, :].astype(NPDT)  # (L, 64)
            for j in range(NKB):
                col = (p * NKB + j) * VW
                va[:, col : col + E] = vp[j * KB : (j + 1) * KB, :]
                va[:, col + E] = 1.0
        im = {"vaug": va, "expb": expb_packed}
        for p in range(PAIRS):
            im[f"qt{p}"] = np.ascontiguousarray(qts[p])
            im[f"kt{p}"] = np.ascontiguousarray(kts[p])
        in_maps.append(im)

    res = run_bass_kernel_spmd(nc, in_maps, list(range(NCORES)), trace=TRACE)
    LAST["results"] = res

    out = np.empty((B, L, H, E), dtype=np.float32)
    for c in range(NCORES):
        ot = res.results[c]["ot"]  # (PAIRS, 65, L)
        for p in range(PAIRS):
            out[p, :, c, :] = (ot[p, :E, :] / ot[p, E : E + 1, :]).T
    return out
